# revision 20
# baseline (speedup 1.0000x reference)
"""EMA scan kernel for Trainium2 (Bass/Tile), 8-core SPMD.

Problem: h_t = (1-a)*y_t + a*h_{t-1}, h_{-1}=0, a=0.9, over y [B=4, S=4096, D=2048] f32.

Sharding: B(4) x D-half(2) -> 8 cores, each core handles a [S=4096, Dc=1024] slab.

Per-core algorithm (exact, matmul-based):
  Split S into 32 blocks of TB=128 rows. For block b:
      h_b = L @ y_b + M1 @ z_{b-1}
  where L[t,j]  = (1-a)*a^(t-j) for t>=j else 0          (in-block causal scan)
        M1[t,j] = (1-a)*a^(t+128-j)                      (previous-block window)
  and z_b = y_b + a^128 * z_{b-1} is a block-level EMA of the raw inputs.
  This is exact because the lag-(128m) window matrices satisfy M_m = a^(128(m-1)) * M1,
  so summing M_m @ y_{b-m} over all m telescopes into M1 @ z_{b-1}.

The L matmul runs in true fp32 (4 cyc/row on the PE). The M1 carry matmul
uses an error-free fp32r split: TRN2's fp32r matmul (1 cyc/row) internally
rounds operands to 11 mantissa bits but is EXACT when operands are already
on that grid, so M1@z = M1h@zh + M1h@zl + M1l@zh with
M1h=round11(M1), M1l=round11(M1-M1h) (host-side consts), zh=round11(z)
(an fp32r-dtype tile write rounds), zl=z-zh. Three 1-cyc/row matmuls
replace one 4-cyc/row fp32 matmul with fp32-level accuracy (HW-measured
maxabs 3.6e-7 vs the fp64 scan, identical to the all-fp32 variant).

The fp32 z-chain runs on DVE, zh rounding-copies on GpSimd, zl residuals
on DVE, PSUM->SBUF copies on ACT. Input DMA is batched 4 blocks (2 MiB)
per transfer; output DMA 2 blocks (1 MiB) issued from the ACT HWDGE ring
so in/out DMA setup overlaps. Dummy warmup matmuls during the first input
DMA hold the PE at full clock (HAM). Cost-model (TimelineSim) predicted
per-core exec: ~114 us against a ~94 us HBM roofline for the 32 MiB/core
of traffic.
"""

import numpy as np

import concourse.bass as bass
import concourse.tile as tile
from concourse import bacc, mybir
from concourse import bass_utils

ALPHA = 0.9
B, S, D = 4, 4096, 2048
NCORES = 8
DC = D // 2          # per-core D chunk (1024)
TB = 128             # S-block size (partition dim)
NB = S // TB         # 32 blocks
GK = 4               # blocks per DMA group
NG = NB // GK        # 8 groups
NC_CHUNK = 512       # matmul moving-operand chunk (one PSUM bank, fp32)
F32 = mybir.dt.float32
F32R = mybir.dt.float32r  # tf32-class PE fast path (1 cyc/row vs 4 for fp32)


def _round11(a):
    # round fp32 to 11 explicit mantissa bits (the fp32r-representable grid;
    # such values pass through fp32r matmuls bit-exactly)
    u = np.ascontiguousarray(a, dtype=np.float32).view(np.uint32)
    u2 = ((u + np.uint32(1 << 11)) >> 12) << 12
    return u2.astype(np.uint32).view(np.float32)


def _consts():
    a = ALPHA
    t = np.arange(TB)
    diff = t[:, None] - t[None, :]
    L = np.where(diff >= 0, (1.0 - a) * a ** np.maximum(diff, 0), 0.0)
    M1 = (1.0 - a) * a ** (t[:, None] + TB - t[None, :])
    LT = np.ascontiguousarray(L.T).astype(np.float32)
    M1T = np.ascontiguousarray(M1.T).astype(np.float32)
    c = float(a**TB)
    M1H = _round11(M1T)
    M1LO = _round11(M1T.astype(np.float64) - M1H.astype(np.float64))
    return LT, M1T, c, M1H, M1LO


_CACHE = {}


def _build(ybufs=4, obufs=5, zbufs=3, psbufs=4, gk=GK, dve_frac=0, warmup=12, zsplit=4, out_gk=2, out_eng='scalar', m1_mode='split', zh_eng='gpsimd'):
    key = (ybufs, obufs, zbufs, psbufs, gk, dve_frac, warmup, zsplit, out_gk, out_eng, m1_mode, zh_eng)
    if key in _CACHE:
        return _CACHE[key]
    _, _, c, _, _ = _consts()
    m1_f32 = m1_mode == 'fp32'
    split = m1_mode == 'split'
    ZDT = F32 if (m1_f32 or split) else F32R
    GKL = gk
    NGL = NB // gk

    nc = bacc.Bacc(
        "TRN2",
        target_bir_lowering=False,
        debug=False,
        enable_asserts=False,
        num_devices=NCORES,
    )
    y_dram = nc.dram_tensor("y", [S, DC], F32, kind="ExternalInput")
    lt_dram = nc.dram_tensor("lt", [TB, TB], F32, kind="ExternalInput")
    m1t_dram = nc.dram_tensor("m1t", [TB, TB], F32, kind="ExternalInput")
    if split:
        m1l_dram = nc.dram_tensor("m1l", [TB, TB], F32, kind="ExternalInput")
    out_dram = nc.dram_tensor("out", [S, DC], F32, kind="ExternalOutput")

    with tile.TileContext(nc) as tc:
        with (
            tc.tile_pool(name="consts", bufs=1) as cpool,
            tc.tile_pool(name="ypool", bufs=ybufs) as ypool,
            tc.tile_pool(name="zpool", bufs=zbufs) as zpool,
            tc.tile_pool(name="opool", bufs=obufs) as opool,
            tc.tile_pool(name="zhpool", bufs=zbufs) as zhpool,
            tc.tile_pool(name="zlpool", bufs=zbufs) as zlpool,
            tc.tile_pool(name="psum", bufs=psbufs, space=bass.MemorySpace.PSUM) as pspool,
            tc.tile_pool(name="wps", bufs=1, space=bass.MemorySpace.PSUM) as wpool,
        ):
            lt_sb = cpool.tile([TB, TB], F32, tag="lt")
            m1t_sb = cpool.tile([TB, TB], F32 if m1_f32 else F32R, tag="m1t")
            nc.sync.dma_start(lt_sb[:], lt_dram[:])
            if m1_f32:
                nc.sync.dma_start(m1t_sb[:], m1t_dram[:])
            else:
                # SWDGE dma casts fp32 -> fp32r (the verifier requires fp32r
                # matmul operands to be produced pre-rounded; m1 consts are
                # pre-rounded host-side so the cast is bit-exact)
                nc.gpsimd.dma_start(m1t_sb[:], m1t_dram[:])
            if split:
                m1l_sb = cpool.tile([TB, TB], F32R, tag="m1l")
                nc.gpsimd.dma_start(m1l_sb[:], m1l_dram[:])

            # PE warmup: dummy matmuls on the const tile while the first
            # y-group DMA is in flight, so real matmuls start at full clock
            # (HAM needs ~3us of continuous PE activity).
            if warmup:
                wps = wpool.tile([TB, TB], F32)
                for _ in range(warmup):
                    nc.tensor.matmul(
                        wps[:], lt_sb[:], lt_sb[:], start=True, stop=True
                    )

            zprev = None
            for g in range(NGL):
                rows = slice(g * GKL * TB, (g + 1) * GKL * TB)
                y_t = ypool.tile([TB, GKL, DC], F32)
                nc.sync.dma_start(
                    y_t[:], y_dram[rows, :].rearrange("(k p) d -> p k d", k=GKL, p=TB)
                )
                ogk = out_gk or GKL
                o_t = None
                for k in range(GKL):
                    if k % ogk == 0:
                        o_t = opool.tile([TB, ogk, DC], F32)
                    ko = k % ogk
                    b = g * GKL + k
                    # block-level EMA of inputs: z_b = y_b + a^128 * z_{b-1}
                    # (split into independent column chunks to shorten the
                    # serial chain; emitted first so DVE dispatches it early)
                    zcur = None
                    if 0 < b < NB - 1:
                        z_t = zpool.tile([TB, DC], ZDT)
                        zw = DC // zsplit
                        for zi in range(zsplit):
                            cols = slice(zi * zw, (zi + 1) * zw)
                            zp = zprev[0] if split else zprev
                            nc.vector.scalar_tensor_tensor(
                                z_t[:, cols],
                                zp[:, cols],
                                c,
                                y_t[:, k, cols],
                                op0=mybir.AluOpType.mult,
                                op1=mybir.AluOpType.add,
                            )
                        zcur = z_t[:]
                    elif b == 0:
                        if split:
                            zcur = y_t[:, 0, :]
                        else:
                            z_t = zpool.tile([TB, DC], ZDT)
                            nc.vector.tensor_copy(z_t[:], y_t[:, 0, :])
                            zcur = z_t[:]
                    if split and zcur is not None and b < NB - 1:
                        # error-free split of z for exact fp32r matmuls:
                        # zh = round11(z) (fp32r write rounds), zl = z - zh.
                        zh_t = zhpool.tile([TB, DC], F32R)
                        zl_t = zlpool.tile([TB, DC], F32R)
                        zh_engine = nc.gpsimd if zh_eng == 'gpsimd' else nc.vector
                        zh_engine.tensor_copy(zh_t[:], zcur)
                        nc.vector.tensor_tensor(
                            zl_t[:], zcur, zh_t[:], op=mybir.AluOpType.subtract
                        )
                        zcur = (zcur, zh_t[:], zl_t[:])
                    for n0 in (0, NC_CHUNK):
                        ps = pspool.tile([TB, NC_CHUNK], F32)
                        rhs_y = y_t[:, k, n0 : n0 + NC_CHUNK]
                        cs = slice(n0, n0 + NC_CHUNK)
                        if b == 0:
                            nc.tensor.matmul(ps[:], lt_sb[:], rhs_y, start=True, stop=True)
                        elif split:
                            zh_p, zl_p = zprev[1], zprev[2]
                            nc.tensor.matmul(ps[:], m1t_sb[:], zh_p[:, cs], start=True, stop=False)
                            nc.tensor.matmul(ps[:], m1t_sb[:], zl_p[:, cs], start=False, stop=False)
                            nc.tensor.matmul(ps[:], m1l_sb[:], zh_p[:, cs], start=False, stop=False)
                            nc.tensor.matmul(ps[:], lt_sb[:], rhs_y, start=False, stop=True)
                        else:
                            # carry matmul in fp32r (tf32-class)
                            nc.tensor.matmul(
                                ps[:], m1t_sb[:], zprev[:, cs], start=True, stop=False
                            )
                            nc.tensor.matmul(ps[:], lt_sb[:], rhs_y, start=False, stop=True)
                        dst = o_t[:, ko, n0 : n0 + NC_CHUNK]
                        if dve_frac and (2 * b + (n0 != 0)) % (dve_frac + 1) < dve_frac:
                            nc.vector.tensor_copy(dst, ps[:])
                        else:
                            nc.scalar.copy(dst, ps[:])
                    if zcur is not None:
                        zprev = zcur
                    if k % ogk == ogk - 1:
                        r0 = (g * GKL + k - ogk + 1) * TB
                        orows = slice(r0, r0 + ogk * TB)
                        out_engine = nc.scalar if out_eng == 'scalar' else nc.sync
                        out_engine.dma_start(
                            out_dram[orows, :].rearrange(
                                "(k p) d -> p k d", k=ogk, p=TB
                            ),
                            o_t[:],
                        )

    nc.compile()
    _CACHE[key] = nc
    return nc


def kernel(y_seq):
    y_seq = np.asarray(y_seq, dtype=np.float32)
    assert y_seq.shape == (B, S, D), y_seq.shape
    LT, M1T, _, M1H, M1LO = _consts()
    nc = _build()

    in_maps = []
    for core in range(NCORES):
        b, h = divmod(core, 2)
        shard = np.ascontiguousarray(y_seq[b, :, h * DC : (h + 1) * DC])
        in_maps.append({"y": shard, "lt": LT, "m1t": M1H, "m1l": M1LO})

    try:
        res = bass_utils.run_bass_kernel_spmd(
            nc, in_maps, core_ids=list(range(NCORES))
        )
    except Exception:
        # transient NRT/device hiccups (e.g. first-exec unrecoverable state)
        # have been observed to succeed on retry
        res = bass_utils.run_bass_kernel_spmd(
            nc, in_maps, core_ids=list(range(NCORES))
        )

    out = np.empty((B, S, D), dtype=np.float32)
    for core in range(NCORES):
        b, h = divmod(core, 2)
        out[b, :, h * DC : (h + 1) * DC] = res.results[core]["out"]
    return out


# revision 23
# speedup vs baseline: 1.0521x; 1.0521x over previous
"""EMA scan kernel for Trainium2 (Bass/Tile), 8-core SPMD.

Problem: h_t = (1-a)*y_t + a*h_{t-1}, h_{-1}=0, a=0.9, over y [B=4, S=4096, D=2048] f32.

Sharding: B(4) x D-half(2) -> 8 cores, each core handles a [S=4096, Dc=1024] slab.

Per-core algorithm (exact, matmul-based):
  Split S into 32 blocks of TB=128 rows. For block b:
      h_b = L @ y_b + M1 @ z_{b-1}
  where L[t,j]  = (1-a)*a^(t-j) for t>=j else 0          (in-block causal scan)
        M1[t,j] = (1-a)*a^(t+128-j)                      (previous-block window)
  and z_b = y_b + a^128 * z_{b-1} is a block-level EMA of the raw inputs.
  This is exact because the lag-(128m) window matrices satisfy M_m = a^(128(m-1)) * M1,
  so summing M_m @ y_{b-m} over all m telescopes into M1 @ z_{b-1}.

The L matmul runs in true fp32 (4 cyc/row on the PE). The M1 carry matmul
uses an error-free fp32r split: TRN2's fp32r matmul (1 cyc/row) internally
rounds operands to 11 mantissa bits but is EXACT when operands are already
on that grid, so M1@z = M1h@zh + M1h@zl + M1l@zh with
M1h=round11(M1), M1l=round11(M1-M1h) (host-side consts), zh=round11(z)
(an fp32r-dtype tile write rounds), zl=z-zh. Three 1-cyc/row matmuls
replace one 4-cyc/row fp32 matmul with fp32-level accuracy (HW-measured
maxabs 3.6e-7 vs the fp64 scan, identical to the all-fp32 variant).

The fp32 z-chain runs on DVE, zh rounding-copies on GpSimd, zl residuals
on DVE, PSUM->SBUF copies on ACT. Input DMA is batched 4 blocks (2 MiB)
per transfer; output DMA 2 blocks (1 MiB) issued from the ACT HWDGE ring
so in/out DMA setup overlaps. Dummy warmup matmuls during the first input
DMA hold the PE at full clock (HAM). Cost-model (TimelineSim) predicted
per-core exec: ~114 us against a ~94 us HBM roofline for the 32 MiB/core
of traffic.
"""

import numpy as np

import concourse.bass as bass
import concourse.tile as tile
from concourse import bacc, mybir
from concourse import bass_utils

ALPHA = 0.9
B, S, D = 4, 4096, 2048
NCORES = 8
DC = D // 2          # per-core D chunk (1024)
TB = 128             # S-block size (partition dim)
NB = S // TB         # 32 blocks
GK = 4               # blocks per DMA group
NG = NB // GK        # 8 groups
NC_CHUNK = 512       # matmul moving-operand chunk (one PSUM bank, fp32)
F32 = mybir.dt.float32
F32R = mybir.dt.float32r  # tf32-class PE fast path (1 cyc/row vs 4 for fp32)


def _round11(a):
    # round fp32 to 11 explicit mantissa bits (the fp32r-representable grid;
    # such values pass through fp32r matmuls bit-exactly)
    u = np.ascontiguousarray(a, dtype=np.float32).view(np.uint32)
    u2 = ((u + np.uint32(1 << 11)) >> 12) << 12
    return u2.astype(np.uint32).view(np.float32)


def _consts():
    a = ALPHA
    t = np.arange(TB)
    diff = t[:, None] - t[None, :]
    L = np.where(diff >= 0, (1.0 - a) * a ** np.maximum(diff, 0), 0.0)
    M1 = (1.0 - a) * a ** (t[:, None] + TB - t[None, :])
    LT = np.ascontiguousarray(L.T).astype(np.float32)
    M1T = np.ascontiguousarray(M1.T).astype(np.float32)
    c = float(a**TB)
    M1H = _round11(M1T)
    M1LO = _round11(M1T.astype(np.float64) - M1H.astype(np.float64))
    return LT, M1T, c, M1H, M1LO


_CACHE = {}


def _build(ybufs=4, obufs=5, zbufs=3, psbufs=4, gk=GK, dve_frac=0, warmup=6, zsplit=4, out_gk=2, out_eng='scalar', m1_mode='split', zh_eng='gpsimd', l_first=True, head2=4, tail1=True):
    key = (ybufs, obufs, zbufs, psbufs, gk, dve_frac, warmup, zsplit, out_gk, out_eng, m1_mode, zh_eng, l_first, head2, tail1)
    if key in _CACHE:
        return _CACHE[key]
    _, _, c, _, _ = _consts()
    m1_f32 = m1_mode == 'fp32'
    split = m1_mode == 'split'
    ZDT = F32 if (m1_f32 or split) else F32R
    GKL = gk
    NGL = NB // gk

    nc = bacc.Bacc(
        "TRN2",
        target_bir_lowering=False,
        debug=False,
        enable_asserts=False,
        num_devices=NCORES,
    )
    y_dram = nc.dram_tensor("y", [S, DC], F32, kind="ExternalInput")
    lt_dram = nc.dram_tensor("lt", [TB, TB], F32, kind="ExternalInput")
    m1t_dram = nc.dram_tensor("m1t", [TB, TB], F32, kind="ExternalInput")
    if split:
        m1l_dram = nc.dram_tensor("m1l", [TB, TB], F32, kind="ExternalInput")
    out_dram = nc.dram_tensor("out", [S, DC], F32, kind="ExternalOutput")

    with tile.TileContext(nc) as tc:
        with (
            tc.tile_pool(name="consts", bufs=1) as cpool,
            tc.tile_pool(name="ypool", bufs=ybufs) as ypool,
            tc.tile_pool(name="zpool", bufs=zbufs) as zpool,
            tc.tile_pool(name="opool", bufs=obufs) as opool,
            tc.tile_pool(name="zhpool", bufs=zbufs) as zhpool,
            tc.tile_pool(name="zlpool", bufs=zbufs) as zlpool,
            tc.tile_pool(name="psum", bufs=psbufs, space=bass.MemorySpace.PSUM) as pspool,
            tc.tile_pool(name="wps", bufs=1, space=bass.MemorySpace.PSUM) as wpool,
        ):
            lt_sb = cpool.tile([TB, TB], F32, tag="lt")
            m1t_sb = cpool.tile([TB, TB], F32 if m1_f32 else F32R, tag="m1t")
            nc.sync.dma_start(lt_sb[:], lt_dram[:])
            if m1_f32:
                nc.sync.dma_start(m1t_sb[:], m1t_dram[:])
            else:
                # SWDGE dma casts fp32 -> fp32r (the verifier requires fp32r
                # matmul operands to be produced pre-rounded; m1 consts are
                # pre-rounded host-side so the cast is bit-exact)
                nc.gpsimd.dma_start(m1t_sb[:], m1t_dram[:])
            if split:
                m1l_sb = cpool.tile([TB, TB], F32R, tag="m1l")
                nc.gpsimd.dma_start(m1l_sb[:], m1l_dram[:])

            # PE warmup: dummy matmuls on the const tile while the first
            # y-group DMA is in flight, so real matmuls start at full clock
            # (HAM needs ~3us of continuous PE activity).
            if warmup:
                wps = wpool.tile([TB, TB], F32)
                for _ in range(warmup):
                    nc.tensor.matmul(
                        wps[:], lt_sb[:], lt_sb[:], start=True, stop=True
                    )

            zprev = None
            ko_acc = 0
            group_sizes = [2] * head2 + [GKL] * ((NB - 2 * head2) // GKL)
            assert sum(group_sizes) == NB
            gstart = 0
            for g, gsz in enumerate(group_sizes):
                rows = slice(gstart * TB, (gstart + gsz) * TB)
                y_t = ypool.tile([TB, gsz, DC], F32, tag="y_t")
                nc.sync.dma_start(
                    y_t[:], y_dram[rows, :].rearrange("(k p) d -> p k d", k=gsz, p=TB)
                )
                ogk = min(out_gk or gsz, gsz)
                o_t = None
                for k in range(gsz):
                    b = gstart + k
                    cur_ogk = 1 if (tail1 and b >= NB - 2) else ogk
                    if ko_acc == 0:
                        o_t = opool.tile([TB, cur_ogk, DC], F32, tag="o_t")
                    ko = ko_acc
                    # block-level EMA of inputs: z_b = y_b + a^128 * z_{b-1}
                    # (split into independent column chunks to shorten the
                    # serial chain; emitted first so DVE dispatches it early)
                    zcur = None
                    if 0 < b < NB - 1:
                        z_t = zpool.tile([TB, DC], ZDT)
                        zw = DC // zsplit
                        for zi in range(zsplit):
                            cols = slice(zi * zw, (zi + 1) * zw)
                            zp = zprev[0] if split else zprev
                            nc.vector.scalar_tensor_tensor(
                                z_t[:, cols],
                                zp[:, cols],
                                c,
                                y_t[:, k, cols],
                                op0=mybir.AluOpType.mult,
                                op1=mybir.AluOpType.add,
                            )
                        zcur = z_t[:]
                    elif b == 0:
                        if split:
                            zcur = y_t[:, 0, :]
                        else:
                            z_t = zpool.tile([TB, DC], ZDT)
                            nc.vector.tensor_copy(z_t[:], y_t[:, 0, :])
                            zcur = z_t[:]
                    if split and zcur is not None and b < NB - 1:
                        # error-free split of z for exact fp32r matmuls:
                        # zh = round11(z) (fp32r write rounds), zl = z - zh.
                        # Split into matmul-chunk halves so each chunk's carry
                        # matmuls start as soon as its half is ready; alternate
                        # zh halves across GpSimd/ACT (both otherwise idle-ish).
                        zh_t = zhpool.tile([TB, DC], F32R)
                        zl_t = zlpool.tile([TB, DC], F32R)
                        for hi, h0 in enumerate((0, NC_CHUNK)):
                            hs = slice(h0, h0 + NC_CHUNK)
                            if zh_eng == 'gpsimd':
                                zh_engine = nc.gpsimd
                            elif zh_eng == 'act':
                                zh_engine = nc.scalar
                            else:
                                zh_engine = nc.gpsimd if hi == 0 else nc.scalar
                            if zh_engine is nc.scalar:
                                zh_engine.copy(zh_t[:, hs], zcur[:, hs])
                            else:
                                zh_engine.tensor_copy(zh_t[:, hs], zcur[:, hs])
                            nc.vector.tensor_tensor(
                                zl_t[:, hs],
                                zcur[:, hs],
                                zh_t[:, hs],
                                op=mybir.AluOpType.subtract,
                            )
                        zcur = (zcur, zh_t[:], zl_t[:])
                    for n0 in (0, NC_CHUNK):
                        ps = pspool.tile([TB, NC_CHUNK], F32)
                        rhs_y = y_t[:, k, n0 : n0 + NC_CHUNK]
                        cs = slice(n0, n0 + NC_CHUNK)
                        if b == 0:
                            nc.tensor.matmul(ps[:], lt_sb[:], rhs_y, start=True, stop=True)
                        elif split:
                            zh_p, zl_p = zprev[1], zprev[2]
                            if l_first:
                                nc.tensor.matmul(ps[:], lt_sb[:], rhs_y, start=True, stop=False)
                                nc.tensor.matmul(ps[:], m1t_sb[:], zh_p[:, cs], start=False, stop=False)
                                nc.tensor.matmul(ps[:], m1t_sb[:], zl_p[:, cs], start=False, stop=False)
                                nc.tensor.matmul(ps[:], m1l_sb[:], zh_p[:, cs], start=False, stop=True)
                            else:
                                nc.tensor.matmul(ps[:], m1t_sb[:], zh_p[:, cs], start=True, stop=False)
                                nc.tensor.matmul(ps[:], m1t_sb[:], zl_p[:, cs], start=False, stop=False)
                                nc.tensor.matmul(ps[:], m1l_sb[:], zh_p[:, cs], start=False, stop=False)
                                nc.tensor.matmul(ps[:], lt_sb[:], rhs_y, start=False, stop=True)
                        else:
                            # carry matmul in fp32r (tf32-class)
                            nc.tensor.matmul(
                                ps[:], m1t_sb[:], zprev[:, cs], start=True, stop=False
                            )
                            nc.tensor.matmul(ps[:], lt_sb[:], rhs_y, start=False, stop=True)
                        dst = o_t[:, ko, n0 : n0 + NC_CHUNK]
                        if dve_frac and (2 * b + (n0 != 0)) % (dve_frac + 1) < dve_frac:
                            nc.vector.tensor_copy(dst, ps[:])
                        else:
                            nc.scalar.copy(dst, ps[:])
                    if zcur is not None:
                        zprev = zcur
                    ko_acc += 1
                    if ko_acc == cur_ogk:
                        r0 = (b - cur_ogk + 1) * TB
                        orows = slice(r0, r0 + cur_ogk * TB)
                        out_engine = nc.scalar if out_eng == 'scalar' else nc.sync
                        out_engine.dma_start(
                            out_dram[orows, :].rearrange(
                                "(k p) d -> p k d", k=cur_ogk, p=TB
                            ),
                            o_t[:],
                        )
                        ko_acc = 0
                gstart += gsz

    nc.compile()
    _CACHE[key] = nc
    return nc


def kernel(y_seq):
    y_seq = np.asarray(y_seq, dtype=np.float32)
    assert y_seq.shape == (B, S, D), y_seq.shape
    LT, M1T, _, M1H, M1LO = _consts()
    nc = _build()

    in_maps = []
    for core in range(NCORES):
        b, h = divmod(core, 2)
        shard = np.ascontiguousarray(y_seq[b, :, h * DC : (h + 1) * DC])
        in_maps.append({"y": shard, "lt": LT, "m1t": M1H, "m1l": M1LO})

    try:
        res = bass_utils.run_bass_kernel_spmd(
            nc, in_maps, core_ids=list(range(NCORES))
        )
    except Exception:
        # transient NRT/device hiccups (e.g. first-exec unrecoverable state)
        # have been observed to succeed on retry
        res = bass_utils.run_bass_kernel_spmd(
            nc, in_maps, core_ids=list(range(NCORES))
        )

    out = np.empty((B, S, D), dtype=np.float32)
    for core in range(NCORES):
        b, h = divmod(core, 2)
        out[b, :, h * DC : (h + 1) * DC] = res.results[core]["out"]
    return out


# revision 26
# speedup vs baseline: 1.1116x; 1.0566x over previous
"""EMA scan kernel for Trainium2 (Bass/Tile), 8-core SPMD.

Problem: h_t = (1-a)*y_t + a*h_{t-1}, h_{-1}=0, a=0.9, over y [B=4, S=4096, D=2048] f32.

Sharding: B(4) x D-half(2) -> 8 cores, each core handles a [S=4096, Dc=1024] slab.

Per-core algorithm (exact, matmul-based):
  Split S into 32 blocks of TB=128 rows. For block b:
      h_b = L @ y_b + M1 @ z_{b-1}
  where L[t,j]  = (1-a)*a^(t-j) for t>=j else 0          (in-block causal scan)
        M1[t,j] = (1-a)*a^(t+128-j)                      (previous-block window)
  and z_b = y_b + a^128 * z_{b-1} is a block-level EMA of the raw inputs.
  This is exact because the lag-(128m) window matrices satisfy M_m = a^(128(m-1)) * M1,
  so summing M_m @ y_{b-m} over all m telescopes into M1 @ z_{b-1}.

The L matmul runs in true fp32 (4 cyc/row on the PE). The M1 carry matmul
uses an error-free fp32r split: TRN2's fp32r matmul (1 cyc/row) internally
rounds operands to 11 mantissa bits but is EXACT when operands are already
on that grid, so M1@z = M1h@zh + M1h@zl + M1l@zh with
M1h=round11(M1), M1l=round11(M1-M1h) (host-side consts), zh=round11(z)
(an fp32r-dtype tile write rounds), zl=z-zh. Three 1-cyc/row matmuls
replace one 4-cyc/row fp32 matmul with fp32-level accuracy (HW-measured
maxabs 3.6e-7 vs the fp64 scan, identical to the all-fp32 variant).

The fp32 z-chain runs on DVE, zh rounding-copies on GpSimd, zl residuals
on DVE, PSUM->SBUF copies on ACT. Input DMA is batched 4 blocks (2 MiB)
per transfer; output DMA 2 blocks (1 MiB) issued from the ACT HWDGE ring
so in/out DMA setup overlaps. Dummy warmup matmuls during the first input
DMA hold the PE at full clock (HAM). Cost-model (TimelineSim) predicted
per-core exec: ~114 us against a ~94 us HBM roofline for the 32 MiB/core
of traffic.
"""

import numpy as np

import concourse.bass as bass
import concourse.tile as tile
from concourse import bacc, mybir
from concourse import bass_utils

ALPHA = 0.9
B, S, D = 4, 4096, 2048
NCORES = 8
DC = D // 2          # per-core D chunk (1024)
TB = 128             # S-block size (partition dim)
NB = S // TB         # 32 blocks
GK = 4               # blocks per DMA group
NG = NB // GK        # 8 groups
NC_CHUNK = 512       # matmul moving-operand chunk (one PSUM bank, fp32)
F32 = mybir.dt.float32
F32R = mybir.dt.float32r  # tf32-class PE fast path (1 cyc/row vs 4 for fp32)


def _round11(a):
    # round fp32 to 11 explicit mantissa bits (the fp32r-representable grid;
    # such values pass through fp32r matmuls bit-exactly)
    u = np.ascontiguousarray(a, dtype=np.float32).view(np.uint32)
    u2 = ((u + np.uint32(1 << 11)) >> 12) << 12
    return u2.astype(np.uint32).view(np.float32)


def _consts():
    a = ALPHA
    t = np.arange(TB)
    diff = t[:, None] - t[None, :]
    L = np.where(diff >= 0, (1.0 - a) * a ** np.maximum(diff, 0), 0.0)
    M1 = (1.0 - a) * a ** (t[:, None] + TB - t[None, :])
    LT = np.ascontiguousarray(L.T).astype(np.float32)
    M1T = np.ascontiguousarray(M1.T).astype(np.float32)
    c = float(a**TB)
    M1H = _round11(M1T)
    M1LO = _round11(M1T.astype(np.float64) - M1H.astype(np.float64))
    return LT, M1T, c, M1H, M1LO


def _consts2():
    # split2 weight set: h_b = L@z_b + (M1 - c*L)@z_{b-1}, all fp32r via
    # error-free 11-bit splits of both weights and z operands.
    LT, M1T, c, _, _ = _consts()
    LH = _round11(LT)
    LL = _round11(LT.astype(np.float64) - LH.astype(np.float64))
    M1P = M1T.astype(np.float64) - c * LT.astype(np.float64)
    M1PH = _round11(M1P.astype(np.float32))
    M1PL = _round11(M1P - M1PH.astype(np.float64))
    return LH, LL, M1PH, M1PL, c


_CACHE = {}


def _build(ybufs=4, obufs=5, zbufs=5, psbufs=4, gk=GK, dve_frac=0, warmup=6, zsplit=4, out_gk=2, out_eng='scalar', m1_mode='split2', zh_eng='gpsimd', l_first=True, head2=4, tail1=True):
    key = (ybufs, obufs, zbufs, psbufs, gk, dve_frac, warmup, zsplit, out_gk, out_eng, m1_mode, zh_eng, l_first, head2, tail1)
    if key in _CACHE:
        return _CACHE[key]
    _, _, c, _, _ = _consts()
    m1_f32 = m1_mode == 'fp32'
    split = m1_mode == 'split'
    split2 = m1_mode == 'split2'
    ZDT = F32 if (m1_f32 or split or split2) else F32R
    GKL = gk
    NGL = NB // gk

    nc = bacc.Bacc(
        "TRN2",
        target_bir_lowering=False,
        debug=False,
        enable_asserts=False,
        num_devices=NCORES,
    )
    y_dram = nc.dram_tensor("y", [S, DC], F32, kind="ExternalInput")
    lt_dram = nc.dram_tensor("lt", [TB, TB], F32, kind="ExternalInput")
    m1t_dram = nc.dram_tensor("m1t", [TB, TB], F32, kind="ExternalInput")
    if split:
        m1l_dram = nc.dram_tensor("m1l", [TB, TB], F32, kind="ExternalInput")
    if split2:
        ll_dram = nc.dram_tensor("ll", [TB, TB], F32, kind="ExternalInput")
        m1l_dram = nc.dram_tensor("m1l", [TB, TB], F32, kind="ExternalInput")
    out_dram = nc.dram_tensor("out", [S, DC], F32, kind="ExternalOutput")

    with tile.TileContext(nc) as tc:
        with (
            tc.tile_pool(name="consts", bufs=1) as cpool,
            tc.tile_pool(name="ypool", bufs=ybufs) as ypool,
            tc.tile_pool(name="zpool", bufs=zbufs) as zpool,
            tc.tile_pool(name="opool", bufs=obufs) as opool,
            tc.tile_pool(name="zhpool", bufs=zbufs) as zhpool,
            tc.tile_pool(name="zlpool", bufs=zbufs) as zlpool,
            tc.tile_pool(name="psum", bufs=psbufs, space=bass.MemorySpace.PSUM) as pspool,
            tc.tile_pool(name="wps", bufs=1, space=bass.MemorySpace.PSUM) as wpool,
        ):
            lt_sb = cpool.tile([TB, TB], F32R if split2 else F32, tag="lt")
            m1t_sb = cpool.tile([TB, TB], F32 if m1_f32 else F32R, tag="m1t")
            if split2:
                nc.gpsimd.dma_start(lt_sb[:], lt_dram[:])
            else:
                nc.sync.dma_start(lt_sb[:], lt_dram[:])
            if m1_f32:
                nc.sync.dma_start(m1t_sb[:], m1t_dram[:])
            else:
                # SWDGE dma casts fp32 -> fp32r (the verifier requires fp32r
                # matmul operands to be produced pre-rounded; m1 consts are
                # pre-rounded host-side so the cast is bit-exact)
                nc.gpsimd.dma_start(m1t_sb[:], m1t_dram[:])
            if split or split2:
                m1l_sb = cpool.tile([TB, TB], F32R, tag="m1l")
                nc.gpsimd.dma_start(m1l_sb[:], m1l_dram[:])
            if split2:
                ll_sb = cpool.tile([TB, TB], F32R, tag="ll")
                nc.gpsimd.dma_start(ll_sb[:], ll_dram[:])

            # PE warmup: dummy matmuls on the const tile while the first
            # y-group DMA is in flight, so real matmuls start at full clock
            # (HAM needs ~3us of continuous PE activity).
            if warmup:
                wps = wpool.tile([TB, TB], F32)
                for _ in range(warmup):
                    nc.tensor.matmul(
                        wps[:], lt_sb[:], lt_sb[:], start=True, stop=True
                    )

            zprev = None
            ko_acc = 0
            group_sizes = [2] * head2 + [GKL] * ((NB - 2 * head2) // GKL)
            assert sum(group_sizes) == NB
            gstart = 0
            for g, gsz in enumerate(group_sizes):
                rows = slice(gstart * TB, (gstart + gsz) * TB)
                y_t = ypool.tile([TB, gsz, DC], F32, tag="y_t")
                nc.sync.dma_start(
                    y_t[:], y_dram[rows, :].rearrange("(k p) d -> p k d", k=gsz, p=TB)
                )
                ogk = min(out_gk or gsz, gsz)
                o_t = None
                for k in range(gsz):
                    b = gstart + k
                    cur_ogk = 1 if (tail1 and b >= NB - 2) else ogk
                    if ko_acc == 0:
                        o_t = opool.tile([TB, cur_ogk, DC], F32, tag="o_t")
                    ko = ko_acc
                    # block-level EMA of inputs: z_b = y_b + a^128 * z_{b-1}
                    # (split into independent column chunks to shorten the
                    # serial chain; emitted first so DVE dispatches it early)
                    zcur = None
                    if 0 < b < (NB if split2 else NB - 1):
                        z_t = zpool.tile([TB, DC], ZDT)
                        zw = DC // zsplit
                        for zi in range(zsplit):
                            cols = slice(zi * zw, (zi + 1) * zw)
                            zp = zprev[0] if (split or split2) else zprev
                            nc.vector.scalar_tensor_tensor(
                                z_t[:, cols],
                                zp[:, cols],
                                c,
                                y_t[:, k, cols],
                                op0=mybir.AluOpType.mult,
                                op1=mybir.AluOpType.add,
                            )
                        zcur = z_t[:]
                    elif b == 0:
                        if split or split2:
                            zcur = y_t[:, 0, :]
                        else:
                            z_t = zpool.tile([TB, DC], ZDT)
                            nc.vector.tensor_copy(z_t[:], y_t[:, 0, :])
                            zcur = z_t[:]
                    if (split or split2) and zcur is not None and (split2 or b < NB - 1):
                        # error-free split of z for exact fp32r matmuls:
                        # zh = round11(z) (fp32r write rounds), zl = z - zh.
                        # Split into matmul-chunk halves so each chunk's carry
                        # matmuls start as soon as its half is ready; alternate
                        # zh halves across GpSimd/ACT (both otherwise idle-ish).
                        zh_t = zhpool.tile([TB, DC], F32R)
                        zl_t = zlpool.tile([TB, DC], F32R)
                        for hi, h0 in enumerate((0, NC_CHUNK)):
                            hs = slice(h0, h0 + NC_CHUNK)
                            if zh_eng == 'gpsimd':
                                zh_engine = nc.gpsimd
                            elif zh_eng == 'act':
                                zh_engine = nc.scalar
                            else:
                                zh_engine = nc.gpsimd if hi == 0 else nc.scalar
                            if zh_engine is nc.scalar:
                                zh_engine.copy(zh_t[:, hs], zcur[:, hs])
                            else:
                                zh_engine.tensor_copy(zh_t[:, hs], zcur[:, hs])
                            nc.vector.tensor_tensor(
                                zl_t[:, hs],
                                zcur[:, hs],
                                zh_t[:, hs],
                                op=mybir.AluOpType.subtract,
                            )
                        zcur = (zcur, zh_t[:], zl_t[:])
                    for n0 in (0, NC_CHUNK):
                        ps = pspool.tile([TB, NC_CHUNK], F32)
                        rhs_y = y_t[:, k, n0 : n0 + NC_CHUNK]
                        cs = slice(n0, n0 + NC_CHUNK)
                        if split2:
                            zh_c, zl_c = zcur[1], zcur[2]
                            if b == 0:
                                nc.tensor.matmul(ps[:], lt_sb[:], zh_c[:, cs], start=True, stop=False)
                                nc.tensor.matmul(ps[:], lt_sb[:], zl_c[:, cs], start=False, stop=False)
                                nc.tensor.matmul(ps[:], ll_sb[:], zh_c[:, cs], start=False, stop=True)
                            else:
                                zh_p, zl_p = zprev[1], zprev[2]
                                nc.tensor.matmul(ps[:], m1t_sb[:], zh_p[:, cs], start=True, stop=False)
                                nc.tensor.matmul(ps[:], m1t_sb[:], zl_p[:, cs], start=False, stop=False)
                                nc.tensor.matmul(ps[:], m1l_sb[:], zh_p[:, cs], start=False, stop=False)
                                nc.tensor.matmul(ps[:], lt_sb[:], zh_c[:, cs], start=False, stop=False)
                                nc.tensor.matmul(ps[:], lt_sb[:], zl_c[:, cs], start=False, stop=False)
                                nc.tensor.matmul(ps[:], ll_sb[:], zh_c[:, cs], start=False, stop=True)
                        elif b == 0:
                            nc.tensor.matmul(ps[:], lt_sb[:], rhs_y, start=True, stop=True)
                        elif split:
                            zh_p, zl_p = zprev[1], zprev[2]
                            if l_first:
                                nc.tensor.matmul(ps[:], lt_sb[:], rhs_y, start=True, stop=False)
                                nc.tensor.matmul(ps[:], m1t_sb[:], zh_p[:, cs], start=False, stop=False)
                                nc.tensor.matmul(ps[:], m1t_sb[:], zl_p[:, cs], start=False, stop=False)
                                nc.tensor.matmul(ps[:], m1l_sb[:], zh_p[:, cs], start=False, stop=True)
                            else:
                                nc.tensor.matmul(ps[:], m1t_sb[:], zh_p[:, cs], start=True, stop=False)
                                nc.tensor.matmul(ps[:], m1t_sb[:], zl_p[:, cs], start=False, stop=False)
                                nc.tensor.matmul(ps[:], m1l_sb[:], zh_p[:, cs], start=False, stop=False)
                                nc.tensor.matmul(ps[:], lt_sb[:], rhs_y, start=False, stop=True)
                        else:
                            # carry matmul in fp32r (tf32-class)
                            nc.tensor.matmul(
                                ps[:], m1t_sb[:], zprev[:, cs], start=True, stop=False
                            )
                            nc.tensor.matmul(ps[:], lt_sb[:], rhs_y, start=False, stop=True)
                        dst = o_t[:, ko, n0 : n0 + NC_CHUNK]
                        if dve_frac and (2 * b + (n0 != 0)) % (dve_frac + 1) < dve_frac:
                            nc.vector.tensor_copy(dst, ps[:])
                        else:
                            nc.scalar.copy(dst, ps[:])
                    if zcur is not None:
                        zprev = zcur
                    ko_acc += 1
                    if ko_acc == cur_ogk:
                        r0 = (b - cur_ogk + 1) * TB
                        orows = slice(r0, r0 + cur_ogk * TB)
                        out_engine = nc.scalar if out_eng == 'scalar' else nc.sync
                        out_engine.dma_start(
                            out_dram[orows, :].rearrange(
                                "(k p) d -> p k d", k=cur_ogk, p=TB
                            ),
                            o_t[:],
                        )
                        ko_acc = 0
                gstart += gsz

    nc.compile()
    _CACHE[key] = nc
    return nc


def kernel(y_seq):
    y_seq = np.asarray(y_seq, dtype=np.float32)
    assert y_seq.shape == (B, S, D), y_seq.shape
    LH, LL, M1PH, M1PL, _ = _consts2()
    nc = _build()

    in_maps = []
    for core in range(NCORES):
        b, h = divmod(core, 2)
        shard = np.ascontiguousarray(y_seq[b, :, h * DC : (h + 1) * DC])
        in_maps.append(
            {"y": shard, "lt": LH, "ll": LL, "m1t": M1PH, "m1l": M1PL}
        )

    try:
        res = bass_utils.run_bass_kernel_spmd(
            nc, in_maps, core_ids=list(range(NCORES))
        )
    except Exception:
        # transient NRT/device hiccups (e.g. first-exec unrecoverable state)
        # have been observed to succeed on retry
        res = bass_utils.run_bass_kernel_spmd(
            nc, in_maps, core_ids=list(range(NCORES))
        )

    out = np.empty((B, S, D), dtype=np.float32)
    for core in range(NCORES):
        b, h = divmod(core, 2)
        out[b, :, h * DC : (h + 1) * DC] = res.results[core]["out"]
    return out
